# revision 1
# baseline (speedup 1.0000x reference)
"""Trainium2 Bass kernel for nn_CELoss_4896262717859.

Computes, for each query column c = idx_node[k] of a sparse adjacency matrix
(diagonal zeroed), a cross-entropy-style loss over the "lower" (r < c) and
"upper" (r > c) neighbor sets:

    contrib_side(c) = [cnt>0 and poscnt==1] * (log(sum_r m exp(out_r)) - poslogit) / cnt

All per-column quantities are sums of the form sum_r adj[r,c] * w[r] for
w in {1, pos, pos*out, exp(out)} -> computed as tensor-engine matvecs with a
triangular split, per-column for ALL N columns, then gathered at idx_node on
the host (O(N+K) combine).

Sharding: columns split into 8 slabs of 1024 (one per core). Each core reads
its [8192 x 1024] int32 slab contiguously (memory roofline), casts to bf16,
and accumulates psum[12, 1024] stats = {L,U} x {ones, pos, pl_hi, pl_lo,
e_hi, e_lo}. The core's row order is rotated by 1024*core so the diagonal
block always falls in local row-tiles 0..7 -> one NEFF serves all cores; the
L/U routing of full tiles is data-driven via zero-padded weight variants.
"""

import numpy as np
import ml_dtypes

N = 8192
K = 4096
NCORES = 8
SLAB = N // NCORES        # 1024 columns per core
P = 128                   # partition / tile edge
NT = N // P               # 64 row tiles
TPC = SLAB // P           # 8 diagonal tiles per core
NW = 6                    # weights per side
M = 2 * NW                # 12 psum partitions (L half = 0:6, U half = 6:12)
MMN = 512                 # max matmul free size

BF16 = ml_dtypes.bfloat16

_BASS_CACHE = {}


def _build_bass():
    import concourse.tile as tile
    import concourse.mybir as mybir
    from concourse import bacc

    # Bacc (not raw Bass): its compile() runs generate_event_semaphores,
    # which splits multi-sem waits — TRN2 instructions hold at most one.
    nc = bacc.Bacc("TRN2")
    adj = nc.dram_tensor("adj", [N, SLAB], mybir.dt.int32, kind="ExternalInput")
    wmat = nc.dram_tensor(
        "wmat", [P, (NT + TPC) * M], mybir.dt.bfloat16, kind="ExternalInput"
    )
    masks = nc.dram_tensor("masks", [P, 2 * P], mybir.dt.bfloat16, kind="ExternalInput")
    stats = nc.dram_tensor("stats", [M, SLAB], mybir.dt.float32, kind="ExternalOutput")

    with tile.TileContext(nc) as tc:
        with (
            tc.tile_pool(name="singles", bufs=1) as singles,
            # bufs multiple of 8 matches the 8-queue HWDGE round-robin: the
            # slot-reuse predecessor of each adj DMA lands on the SAME queue,
            # so its WAW ordering is implicit and the DMA carries a single
            # sync-wait (the DMA ISA struct has room for only one).
            tc.tile_pool(name="io", bufs=8) as io_pool,
            tc.tile_pool(name="bf", bufs=6) as bf_pool,
            tc.tile_pool(name="diag", bufs=TPC) as diag_pool,
            tc.tile_pool(name="psum", bufs=1, space="PSUM") as psum_pool,
        ):
            # issue the first two adjacency DMAs before anything else so the
            # HBM-saturated stream (the critical path) starts ~1.3us earlier;
            # the small wmat/masks loads slot in behind them.
            pre = {}
            for j in range(2):
                t = io_pool.tile([P, SLAB], mybir.dt.int32, tag="adj_i")
                nc.sync.dma_start(out=t, in_=adj[j * P : (j + 1) * P, :])
                pre[j] = t

            wsb = singles.tile([P, (NT + TPC) * M], mybir.dt.bfloat16)
            nc.sync.dma_start(out=wsb, in_=wmat[:, :])
            msb_raw = singles.tile([P, 2 * P], mybir.dt.bfloat16)
            nc.sync.dma_start(out=msb_raw, in_=masks[:, :])
            # Re-produce the masks on DVE: the DVE TensorTensor ISA struct has
            # room for a single sync-wait, so the diag-mask multiplies must
            # only ever depend on DVE-produced operands (one self-sem wait).
            msb = singles.tile([P, 2 * P], mybir.dt.bfloat16)
            nc.vector.tensor_copy(msb, msb_raw)

            # one psum tile per 512-col bank: Tile's RAW deps are whole-tile,
            # so separate tiles let bank A's copy-out overlap bank B's final
            # matmuls
            accs = [
                psum_pool.tile(
                    [M, MMN], mybir.dt.float32, tag=f"acc{b}", name=f"acc{b}"
                )
                for b in range(SLAB // MMN)
            ]

            def wv(v):
                return wsb[:, v * M : (v + 1) * M]

            # start=True zeroes the ENTIRE psum bank(s) a matmul touches, so
            # (a) every matmul stays inside one 512-col bank, (b) exactly the
            # first matmul touching each bank carries start=True.
            bank_started = [False] * (SLAB // MMN)

            def mm_seg(w, rhs_slice, a, b, stop=False):
                bank = a // MMN
                assert b <= (bank + 1) * MMN
                nc.tensor.matmul(
                    accs[bank][:, a - bank * MMN : b - bank * MMN], w, rhs_slice,
                    start=not bank_started[bank], stop=stop,
                    skip_group_check=True,
                )
                bank_started[bank] = True

            def mm(w, rhs_full, a, b, stop=False):
                while a < b:
                    e = min(b, (a // MMN + 1) * MMN)
                    mm_seg(w, rhs_full[:, a:e], a, e, stop=stop)
                    a = e

            for j in range(NT):
                last = j == NT - 1
                if j in pre:
                    adj_i = pre.pop(j)
                else:
                    adj_i = io_pool.tile([P, SLAB], mybir.dt.int32, tag="adj_i")
                    if last:
                        # split the final load so its first half (and the
                        # bank-A matmul) overlaps the second half's transfer
                        nc.sync.dma_start(
                            out=adj_i[:, 0:MMN], in_=adj[j * P :, 0:MMN]
                        )
                        nc.sync.dma_start(
                            out=adj_i[:, MMN:], in_=adj[j * P :, MMN:]
                        )
                    else:
                        nc.sync.dma_start(out=adj_i, in_=adj[j * P : (j + 1) * P, :])
                adj_b = bf_pool.tile([P, SLAB], mybir.dt.bfloat16)
                if last:
                    # fine-grained pipeline on the final tile: shortest
                    # latency from last-byte-arrival to last matmul, with
                    # the final chunk halved again to 128 cols
                    bounds = [0, 256, 512, 768, 896, SLAB]
                    for s, e in zip(bounds[:-1], bounds[1:]):
                        nc.vector.tensor_copy(adj_b[:, s:e], adj_i[:, s:e])
                        mm(wv(j), adj_b, s, e, stop=(e == SLAB))
                    continue
                nc.vector.tensor_copy(adj_b, adj_i)

                if j < TPC:
                    WL, WU = wv(j), wv(NT + j)
                    c0, c1 = j * P, (j + 1) * P
                    mlo = diag_pool.tile([P, P], mybir.dt.bfloat16)
                    nc.vector.tensor_mul(mlo, adj_b[:, c0:c1], msb[:, 0:P])
                    mup = diag_pool.tile([P, P], mybir.dt.bfloat16)
                    nc.vector.tensor_mul(mup, adj_b[:, c0:c1], msb[:, P : 2 * P])
                    # full columns left of the diag block: rows > cols -> U
                    mm(WU, adj_b, 0, c0)
                    mm_seg(WL, mlo, c0, c1)
                    mm_seg(WU, mup, c0, c1)
                    # full columns right of the diag block: rows < cols -> L
                    mm(WL, adj_b, c1, SLAB)
                else:
                    mm(wv(j), adj_b, 0, SLAB, stop=last)

            # per-bank copy-out: bank A's copy/DMA overlap the final bank-B
            # matmul (ACT reads psum bank A while PE writes bank B); bank B's
            # copy is split across ACT and DVE so the two halves run in
            # parallel on the critical tail
            out_sb = singles.tile([M, SLAB], mybir.dt.float32)
            nc.scalar.copy(out_sb[:, 0:MMN], accs[0])
            nc.sync.dma_start(out=stats[:, 0:MMN], in_=out_sb[:, 0:MMN])
            half = MMN // 2
            nc.scalar.copy(out_sb[:, MMN : MMN + half], accs[1][:, 0:half])
            nc.vector.tensor_copy(out_sb[:, MMN + half :], accs[1][:, half:])
            nc.sync.dma_start(out=stats[:, MMN:], in_=out_sb[:, MMN:])

    nc.compile()
    return nc


def _split_bf16(v):
    hi = v.astype(BF16)
    lo = (v - hi.astype(np.float64)).astype(BF16)
    return hi, lo


def _host_prep(outputs, targets):
    """Per-row weight table Wside [N, 6] bf16 and per-core inputs."""
    out = np.asarray(outputs, np.float64).reshape(-1)
    pos = (np.asarray(targets).reshape(-1) != 0).astype(np.float64)
    pl_hi, pl_lo = _split_bf16(pos * out)
    e_hi, e_lo = _split_bf16(np.exp(out))
    wside = np.stack(
        [
            np.ones(N, BF16),
            pos.astype(BF16),
            pl_hi,
            pl_lo,
            e_hi,
            e_lo,
        ],
        axis=1,
    ).astype(BF16)  # [N, 6]

    # triangular masks for the diagonal 128-block (strict)
    ri = np.arange(P)[:, None]
    ci = np.arange(P)[None, :]
    masks = np.concatenate(
        [(ri < ci).astype(BF16), (ri > ci).astype(BF16)], axis=1
    )  # [128, 256]
    return wside, np.ascontiguousarray(masks)


def _build_wmat(wside, core):
    """Per-core weight variants [128, (64+8)*12] bf16.

    Variant j (j<64): weights for local row tile j (absolute tile (8*core+j)%64).
      j < 8  -> L-only variant (diag tiles; U-only twin stored at 64+j)
      j >= 8 -> single variant, L or U half per the tile's position vs the slab
    """
    w = np.zeros((P, NT + TPC, M), dtype=BF16)
    for j in range(NT):
        t = (TPC * core + j) % NT
        rows = wside[t * P : (t + 1) * P, :]  # [128, 6]
        if j < TPC:
            w[:, j, 0:NW] = rows
            w[:, NT + j, NW:M] = rows
        elif j < NT - TPC * core:
            w[:, j, NW:M] = rows  # rows above slab columns -> U
        else:
            w[:, j, 0:NW] = rows  # wrapped rows below slab columns -> L
    return np.ascontiguousarray(w.reshape(P, (NT + TPC) * M))


def _build_shard(node_adj, core):
    """Rotated column slab [N, SLAB] int32: local row rho = (abs_row - SLAB*core) mod N."""
    c0 = SLAB * core
    cols = node_adj[:, c0 : c0 + SLAB]
    if core == 0:
        return np.ascontiguousarray(cols, dtype=np.int32)
    return np.ascontiguousarray(
        np.concatenate([cols[c0:], cols[:c0]], axis=0), dtype=np.int32
    )


def _combine(stats_list, idx_node):
    """stats_list: per-core [12, SLAB] f32 -> scalar loss (f64 math)."""
    full = np.concatenate([np.asarray(s, np.float64) for s in stats_list], axis=1)

    def side_contrib(x):
        cnt, poscnt = x[0], x[1]
        poslogit = x[2] + x[3]
        sumexp = x[4] + x[5]
        valid = (cnt > 0.5) & (np.abs(poscnt - 1.0) < 0.25)
        lse = np.log(np.where(valid, np.maximum(sumexp, 1e-300), 1.0))
        return np.where(valid, (lse - poslogit) / np.maximum(cnt, 1.0), 0.0)

    contrib = side_contrib(full[0:NW]) + side_contrib(full[NW:M])
    idx = np.asarray(idx_node).reshape(-1).astype(np.int64)
    return np.array(contrib[idx].sum(), dtype=np.float32)


def _ensure_axon_hooks_stub():
    """bass_utils imports antenv.axon_hooks when tracing is requested via
    env; the module is absent on some images. Provide a no-op stub so the
    import never crashes (hook=None -> bass_utils skips tracing)."""
    import sys
    import types

    try:
        import antenv.axon_hooks  # noqa: F401
    except ImportError:
        mod = types.ModuleType("antenv.axon_hooks")
        state = {"hook": None}
        mod.set_axon_ntff_profile_hook = lambda h: state.__setitem__("hook", h)
        mod.get_axon_ntff_profile_hook = lambda: state["hook"]
        sys.modules["antenv.axon_hooks"] = mod


def _device_stats(in_maps):
    _ensure_axon_hooks_stub()
    from concourse.bass_utils import run_bass_kernel_spmd

    if "nc" not in _BASS_CACHE:
        _BASS_CACHE["nc"] = _build_bass()
    last_exc = None
    for attempt in range(4):
        try:
            res = run_bass_kernel_spmd(
                _BASS_CACHE["nc"], in_maps, core_ids=list(range(NCORES))
            )
            return [r["stats"] for r in res.results]
        except Exception as e:  # transient NRT/accelerator hiccups
            last_exc = e
            try:
                # a fresh PJRT client usually recovers a transiently
                # "unrecoverable" accelerator; mirrors a process restart
                import jax
                import jax.extend.backend as _jeb

                jax.clear_caches()
                _jeb.clear_backends()
            except Exception:
                pass
            import time

            time.sleep(2.0 * (attempt + 1))
    raise last_exc


def _sim_stats(in_maps):
    """Numpy emulation of the device kernel (same inputs), for logic validation."""
    outs = []
    for m in in_maps:
        adj = m["adj"].astype(np.float32)
        w = m["wmat"].reshape(P, NT + TPC, M).astype(np.float32)
        msk = m["masks"].astype(np.float32)
        lowm, upm = msk[:, 0:P], msk[:, P:]
        acc = np.zeros((M, SLAB), np.float32)
        for j in range(NT):
            tile = adj[j * P : (j + 1) * P, :]
            if j < TPC:
                WL, WU = w[:, j, :], w[:, NT + j, :]
                c0, c1 = j * P, (j + 1) * P
                acc[:, :c0] += WU.T @ tile[:, :c0]
                acc[:, c0:c1] += WL.T @ (tile[:, c0:c1] * lowm)
                acc[:, c0:c1] += WU.T @ (tile[:, c0:c1] * upm)
                acc[:, c1:] += WL.T @ tile[:, c1:]
            else:
                acc += w[:, j, :].T @ tile
        outs.append(acc)
    return outs


def kernel(outputs, targets, node_adj, idx_node, _simulate=False):
    node_adj = np.asarray(node_adj)
    wside, masks = _host_prep(outputs, targets)
    in_maps = [
        {
            "adj": _build_shard(node_adj, d),
            "wmat": _build_wmat(wside, d),
            "masks": masks,
        }
        for d in range(NCORES)
    ]
    stats = _sim_stats(in_maps) if _simulate else _device_stats(in_maps)
    return _combine(stats, idx_node)



# revision 3
# speedup vs baseline: 2.7460x; 2.7460x over previous
"""Trainium2 Bass kernel for nn_CELoss_4896262717859.

For each query column c = idx_node[k] of a sparse adjacency matrix (diagonal
zeroed), a cross-entropy-style loss over the "lower" (r < c) and "upper"
(r >= c) neighbor sets:

    contrib_side(c) = [cnt>0 and poscnt==1] * (lse - poslogit) / cnt

Strategy (v2):
  * Host gathers ONLY the K=4096 needed columns G = node_adj[:, idx_node]
    (diagonal zeroed) -> fp8 (values 0/1, exact), halving device traffic vs
    processing all N columns, and quartering bytes vs int32.
  * Device: per 512-column core slab, 32 fp8 DoubleRow matmuls — each
    covers a PAIR of 128-row tiles at 0.5 PE cycles/column — producing
    per-pair stats [6, 512] = {ones, pos, pl_hi, pl_lo*SC, e_hi, e_lo*SC}
    sums. No masking, no dtype casts, fully static shapes.
  * Host combine: the L/U split boundary (row idx_node[k]) is handled by a
    prefix sum over the 32 per-pair stats plus an exact 256-row partial for
    the boundary pair, then the scalar CE reduction. O(K) work.

Sharding: columns split into 8 slabs of 512 (one per core); every core runs
the identical NEFF (same weights), only the G slab differs.
"""

import numpy as np
import ml_dtypes

N = 8192
K = 4096
NCORES = 8
SLAB = K // NCORES        # 512 columns per core
P = 128                   # partition / tile edge
NT = N // P               # 64 row tiles
PAIRS = NT // 2           # 32 row-tile pairs (DoubleRow granularity)
ROWS_PER_PAIR = 2 * P     # 256
NW = 6                    # stat components per column
SC = 32.0                 # scale for the *_lo fp8 channels
NWP = 16                  # weight inner-dim padding: dual-fp8 ldweights
                          # requires a 16-byte-aligned k-plane stride
CHUNK = 8                 # row tiles per input DMA
NCH = NT // CHUNK         # 8 input DMAs

FP8 = ml_dtypes.float8_e4m3   # == mybir.dt.np(mybir.dt.float8e4); max 240

_BASS_CACHE = {}


def _build_bass():
    import concourse.tile as tile
    import concourse.mybir as mybir
    from concourse import bacc

    nc = bacc.Bacc("TRN2")
    # g[p, j, n] = adjacency row 128*j+p, slab column n (0/1 in fp8)
    g = nc.dram_tensor("g", [P, NT, SLAB], mybir.dt.float8e4, kind="ExternalInput")
    # w[p, j, m] = weight component m for row 128*j+p
    w = nc.dram_tensor("w", [P, NT, NWP], mybir.dt.float8e4, kind="ExternalInput")
    stats = nc.dram_tensor(
        "stats", [NW, PAIRS * SLAB], mybir.dt.float32, kind="ExternalOutput"
    )

    with tile.TileContext(nc) as tc:
        with (
            tc.tile_pool(name="singles", bufs=1) as singles,
            tc.tile_pool(name="io", bufs=NCH) as io_pool,
            tc.tile_pool(name="psum", bufs=8, space="PSUM") as psum_pool,
        ):
            # weights first: the first matmul needs them
            wsb = singles.tile([P, NT, NWP], mybir.dt.float8e4)
            nc.sync.dma_start(out=wsb, in_=w[:, :, :])
            chunks = []
            for c in range(NCH):
                t = io_pool.tile(
                    [P, CHUNK, SLAB], mybir.dt.float8e4, tag="g", name=f"g{c}"
                )
                nc.sync.dma_start(out=t, in_=g[:, c * CHUNK : (c + 1) * CHUNK, :])
                chunks.append(t)

            out_sb = singles.tile([NW, PAIRS * SLAB], mybir.dt.float32)

            for q in range(PAIRS):
                acc = psum_pool.tile(
                    [NW, SLAB], mybir.dt.float32, tag="acc", name=f"acc{q}"
                )
                ch = chunks[(2 * q) // CHUNK]
                off = (2 * q) % CHUNK
                # DoubleRow: lhsT [128, 2, 6], rhs [128, 2, 512] -> out [6, 512]
                # = sum over the two 128-row tiles at 0.5 cycles/column.
                nc.tensor.matmul(
                    acc,
                    wsb[:, 2 * q : 2 * q + 2, 0:NW],
                    ch[:, off : off + 2, :],
                    start=True,
                    stop=True,
                    perf_mode=mybir.MatmulPerfMode.DoubleRow,
                )
                nc.scalar.copy(out_sb[:, q * SLAB : (q + 1) * SLAB], acc)
                if (q + 1) % (PAIRS // 4) == 0:
                    s = (q + 1 - PAIRS // 4) * SLAB
                    e = (q + 1) * SLAB
                    nc.sync.dma_start(out=stats[:, s:e], in_=out_sb[:, s:e])

    nc.compile()
    return nc


def _host_prep(outputs, targets):
    """Quantized weight table [8192, 6] fp8 + exact f64 weights [8192, 4]."""
    out = np.asarray(outputs, np.float64).reshape(-1)
    pos = (np.asarray(targets).reshape(-1) != 0).astype(np.float64)
    # shift exp into fp8 range only if needed (max normal 240 -> ln 240 = 5.48)
    b_shift = max(0.0, float(out.max()) - 4.5)
    pl = pos * out
    ev = np.exp(out - b_shift)

    def split(v):
        hi = v.astype(FP8)
        lo = ((v - hi.astype(np.float64)) * SC).astype(FP8)
        return hi, lo

    pl_hi, pl_lo = split(pl)
    e_hi, e_lo = split(ev)
    wq = np.stack(
        [
            np.ones(N, FP8),
            pos.astype(FP8),
            pl_hi,
            pl_lo,
            e_hi,
            e_lo,
        ],
        axis=1,
    )  # [N, 6] fp8
    wtrue = np.stack([np.ones(N), pos, pl, ev], axis=1)  # [N, 4] f64
    wpad = np.zeros((N, NWP), FP8)
    wpad[:, :NW] = wq
    wmat = np.ascontiguousarray(wpad.reshape(NT, P, NWP).transpose(1, 0, 2))
    return wmat, wtrue, b_shift


def _gather_columns(node_adj, idx_node):
    """G[r, k] = node_adj[r, idx[k]] != 0, diag zeroed. uint8 [N, K]."""
    idx = np.asarray(idx_node).reshape(-1).astype(np.int64)
    G = (np.asarray(node_adj)[:, idx] != 0).astype(np.uint8)
    G[idx, np.arange(K)] = 0  # node_adj[diag] = 0
    return G, idx


def _build_shard(G, core):
    """Per-core [128, 64, 512] fp8 from column slab [N, 512]."""
    cols = G[:, core * SLAB : (core + 1) * SLAB]
    arr = cols.reshape(NT, P, SLAB).transpose(1, 0, 2)
    return np.ascontiguousarray(arr).astype(FP8)


def _combine(stats_list, idx, G, wtrue, b_shift):
    """stats_list: per-core [6, PAIRS*512] f32 -> scalar loss (f64 math)."""
    # Sg[q, m, k]: per-pair stats for all K columns
    Sg = np.empty((PAIRS, NW, K), np.float64)
    for c, s in enumerate(stats_list):
        Sg[:, :, c * SLAB : (c + 1) * SLAB] = (
            np.asarray(s, np.float64).reshape(NW, PAIRS, SLAB).transpose(1, 0, 2)
        )
    C = np.concatenate(
        [np.zeros((1, NW, K)), np.cumsum(Sg, axis=0)], axis=0
    )  # [PAIRS+1, 6, K]

    kk = np.arange(K)
    qk = (idx // ROWS_PER_PAIR).astype(np.int64)
    L_raw = C[qk, :, kk]                        # [K, 6] full pairs below boundary
    U_raw = C[PAIRS, :, kk] - C[qk + 1, :, kk]  # full pairs above boundary

    def unpack(raw):
        cnt = raw[:, 0]
        poscnt = raw[:, 1]
        pl = raw[:, 2] + raw[:, 3] / SC
        ev = raw[:, 4] + raw[:, 5] / SC
        return cnt, poscnt, pl, ev

    # exact f64 partial for the boundary pair (256 rows containing idx[k])
    rows = qk[None, :] * ROWS_PER_PAIR + np.arange(ROWS_PER_PAIR)[:, None]  # [256,K]
    gpair = G[rows, kk[None, :]].astype(np.float64)
    low = (rows < idx[None, :]).astype(np.float64)
    glo = gpair * low
    ghi = gpair - glo

    def partial(gm):
        return [
            gm.sum(axis=0),
            (gm * wtrue[rows, 1]).sum(axis=0),
            (gm * wtrue[rows, 2]).sum(axis=0),
            (gm * wtrue[rows, 3]).sum(axis=0),
        ]

    def side(raw, gm):
        cnt, poscnt, pl, ev = unpack(raw)
        pc, pp, ppl, pe = partial(gm)
        cnt = cnt + pc
        poscnt = poscnt + pp
        pl = pl + ppl
        ev = ev + pe
        valid = (cnt > 0.5) & (np.abs(poscnt - 1.0) < 0.25)
        lse = np.log(np.where(valid, np.maximum(ev, 1e-300), 1.0)) + b_shift
        return np.where(valid, (lse - pl) / np.maximum(cnt, 1.0), 0.0).sum()

    return np.array(side(L_raw, glo) + side(U_raw, ghi), dtype=np.float32)


def _ensure_axon_hooks_stub():
    """bass_utils imports antenv.axon_hooks when tracing is requested via
    env; the module is absent on some images. Provide a no-op stub so the
    import never crashes (hook=None -> bass_utils skips tracing)."""
    import sys
    import types

    try:
        import antenv.axon_hooks  # noqa: F401
    except ImportError:
        mod = types.ModuleType("antenv.axon_hooks")
        state = {"hook": None}
        mod.set_axon_ntff_profile_hook = lambda h: state.__setitem__("hook", h)
        mod.get_axon_ntff_profile_hook = lambda: state["hook"]
        sys.modules["antenv.axon_hooks"] = mod


def _device_stats(in_maps):
    _ensure_axon_hooks_stub()
    from concourse.bass_utils import run_bass_kernel_spmd

    if "nc" not in _BASS_CACHE:
        _BASS_CACHE["nc"] = _build_bass()
    last_exc = None
    for attempt in range(4):
        try:
            res = run_bass_kernel_spmd(
                _BASS_CACHE["nc"], in_maps, core_ids=list(range(NCORES))
            )
            return [r["stats"] for r in res.results]
        except Exception as e:  # transient NRT/accelerator hiccups
            last_exc = e
            try:
                # a fresh PJRT client usually recovers a transiently
                # "unrecoverable" accelerator; mirrors a process restart
                import jax
                import jax.extend.backend as _jeb

                jax.clear_caches()
                _jeb.clear_backends()
            except Exception:
                pass
            import time

            time.sleep(2.0 * (attempt + 1))
    raise last_exc


def _sim_stats(in_maps):
    """Numpy emulation of the device kernel (same inputs), for logic validation."""
    outs = []
    for m in in_maps:
        gm = m["g"].astype(np.float32)     # [128, 64, 512]
        wm = m["w"].astype(np.float32)[:, :, :NW]  # [128, 64, 6]
        acc = np.zeros((NW, PAIRS, SLAB), np.float32)
        for q in range(PAIRS):
            for j in (2 * q, 2 * q + 1):
                acc[:, q, :] += wm[:, j, :].T @ gm[:, j, :]
        outs.append(acc.reshape(NW, PAIRS * SLAB))
    return outs


def _prep(outputs, targets, node_adj, idx_node):
    wmat, wtrue, b_shift = _host_prep(outputs, targets)
    G, idx = _gather_columns(node_adj, idx_node)
    in_maps = [{"g": _build_shard(G, d), "w": wmat} for d in range(NCORES)]
    return in_maps, (idx, G, wtrue, b_shift)


def kernel(outputs, targets, node_adj, idx_node, _simulate=False):
    in_maps, ctx = _prep(outputs, targets, node_adj, idx_node)
    stats = _sim_stats(in_maps) if _simulate else _device_stats(in_maps)
    return _combine(stats, *ctx)


# revision 8
# speedup vs baseline: 3.1296x; 1.1397x over previous
"""Trainium2 Bass kernel for nn_CELoss_4896262717859.

For each query column c = idx_node[k] of a sparse adjacency matrix (diagonal
zeroed), a cross-entropy-style loss over the "lower" (r < c) and "upper"
(r >= c) neighbor sets:

    contrib_side(c) = [cnt>0 and poscnt==1] * (lse - poslogit) / cnt

Strategy (v2):
  * Host gathers ONLY the K=4096 needed columns G = node_adj[:, idx_node]
    (diagonal zeroed) -> fp8 (values 0/1, exact), halving device traffic vs
    processing all N columns, and quartering bytes vs int32.
  * Device: per 512-column core slab, 32 fp8 DoubleRow matmuls — each
    covers a PAIR of 128-row tiles at 0.5 PE cycles/column — producing
    per-pair stats [6, 512] = {ones, pos, pl_hi, pl_lo*SC, e_hi, e_lo*SC}
    sums. No masking, no dtype casts, fully static shapes.
  * Host combine: the L/U split boundary (row idx_node[k]) is handled by a
    prefix sum over the 32 per-pair stats plus an exact 256-row partial for
    the boundary pair, then the scalar CE reduction. O(K) work.

Sharding: columns split into 8 slabs of 512 (one per core); every core runs
the identical NEFF (same weights), only the G slab differs.
"""

import numpy as np
import ml_dtypes

N = 8192
K = 4096
NCORES = 8
SLAB = K // NCORES        # 512 columns per core
P = 128                   # partition / tile edge
NT = N // P               # 64 row tiles
PAIRS = NT // 2           # 32 row-tile pairs (DoubleRow granularity)
ROWS_PER_PAIR = 2 * P     # 256
NW = 6                    # stat components per column
SC = 32.0                 # scale for the *_lo fp8 channels
NWP = 16                  # weight inner-dim padding: dual-fp8 ldweights
                          # requires a 16-byte-aligned k-plane stride
CHUNK = 4                 # row tiles per input DMA
NCH = NT // CHUNK         # 16 input DMAs

FP8 = ml_dtypes.float8_e4m3   # == mybir.dt.np(mybir.dt.float8e4); max 240

_BASS_CACHE = {}


def _build_bass():
    import concourse.tile as tile
    import concourse.mybir as mybir
    from concourse import bacc

    nc = bacc.Bacc("TRN2")
    # g[c, p, u, n] = adjacency row 128*(CHUNK*c+u)+p, slab column n (0/1 fp8).
    # Chunk-major so each chunk DMA reads one fully contiguous DRAM block
    # (strided 4KB reads at 32KB pitch only reach ~260GB/s vs ~356 contiguous).
    g = nc.dram_tensor(
        "g", [NCH, P, CHUNK, SLAB], mybir.dt.float8e4, kind="ExternalInput"
    )
    # w[p, j, m] = weight component m for row 128*j+p
    w = nc.dram_tensor("w", [P, NT, NWP], mybir.dt.float8e4, kind="ExternalInput")
    stats = nc.dram_tensor(
        "stats", [NW, PAIRS * SLAB], mybir.dt.float32, kind="ExternalOutput"
    )

    with tile.TileContext(nc) as tc:
        with (
            tc.tile_pool(name="singles", bufs=1) as singles,
            tc.tile_pool(name="io", bufs=NCH) as io_pool,
            tc.tile_pool(name="psum", bufs=8, space="PSUM") as psum_pool,
        ):
            # weights on the ACT HW queue: sync's queue stays a pure,
            # in-order, full-bandwidth g stream
            wsb = singles.tile([P, NT, NWP], mybir.dt.float8e4)
            nc.scalar.dma_start(out=wsb, in_=w[:, :, :])
            chunks = []
            for c in range(NCH):
                t = io_pool.tile(
                    [P, CHUNK, SLAB], mybir.dt.float8e4, tag="g", name=f"g{c}"
                )
                nc.sync.dma_start(out=t, in_=g[c, :, :, :])
                chunks.append(t)

            out_sb = singles.tile([NW, PAIRS * SLAB], mybir.dt.float32)

            PPC = CHUNK // 2  # pairs per chunk
            for q in range(PAIRS):
                acc = psum_pool.tile(
                    [NW, SLAB], mybir.dt.float32, tag="acc", name=f"acc{q}"
                )
                ch = chunks[(2 * q) // CHUNK]
                off = (2 * q) % CHUNK
                # DoubleRow: lhsT [128, 2, 6], rhs [128, 2, 512] -> out [6, 512]
                # = sum over the two 128-row tiles at 0.5 cycles/column.
                nc.tensor.matmul(
                    acc,
                    wsb[:, 2 * q : 2 * q + 2, 0:NW],
                    ch[:, off : off + 2, :],
                    start=True,
                    stop=True,
                    perf_mode=mybir.MatmulPerfMode.DoubleRow,
                )
                # psum -> sbuf staging alternates DVE/ACT so neither engine's
                # ~570ns copy chain paces the matmul stream
                dst = out_sb[:, q * SLAB : (q + 1) * SLAB]
                if q % 2 == 0:
                    nc.vector.tensor_copy(dst, acc)
                else:
                    nc.scalar.copy(dst, acc)
                if (q + 1) % (PAIRS // 4) == 0:
                    s = (q + 1 - PAIRS // 4) * SLAB
                    e = (q + 1) * SLAB
                    nc.scalar.dma_start(out=stats[:, s:e], in_=out_sb[:, s:e])

    nc.compile()
    return nc


def _host_prep(outputs, targets):
    """Quantized weight table [8192, 6] fp8 + exact f64 weights [8192, 4]."""
    out = np.asarray(outputs, np.float64).reshape(-1)
    pos = (np.asarray(targets).reshape(-1) != 0).astype(np.float64)
    # shift exp into fp8 range only if needed (max normal 240 -> ln 240 = 5.48)
    b_shift = max(0.0, float(out.max()) - 4.5)
    pl = pos * out
    ev = np.exp(out - b_shift)

    def split(v):
        hi = v.astype(FP8)
        lo = ((v - hi.astype(np.float64)) * SC).astype(FP8)
        return hi, lo

    pl_hi, pl_lo = split(pl)
    e_hi, e_lo = split(ev)
    wq = np.stack(
        [
            np.ones(N, FP8),
            pos.astype(FP8),
            pl_hi,
            pl_lo,
            e_hi,
            e_lo,
        ],
        axis=1,
    )  # [N, 6] fp8
    wtrue = np.stack([np.ones(N), pos, pl, ev], axis=1)  # [N, 4] f64
    wpad = np.zeros((N, NWP), FP8)
    wpad[:, :NW] = wq
    wmat = np.ascontiguousarray(wpad.reshape(NT, P, NWP).transpose(1, 0, 2))
    return wmat, wtrue, b_shift


def _gather_columns(node_adj, idx_node):
    """G[r, k] = node_adj[r, idx[k]] != 0, diag zeroed. uint8 [N, K]."""
    idx = np.asarray(idx_node).reshape(-1).astype(np.int64)
    G = (np.asarray(node_adj)[:, idx] != 0).astype(np.uint8)
    G[idx, np.arange(K)] = 0  # node_adj[diag] = 0
    return G, idx


def _build_shard(G, core):
    """Per-core [NCH, 128, CHUNK, 512] fp8 from column slab [N, 512]."""
    cols = G[:, core * SLAB : (core + 1) * SLAB]
    arr = cols.reshape(NCH, CHUNK, P, SLAB).transpose(0, 2, 1, 3)
    return np.ascontiguousarray(arr).astype(FP8)


def _combine(stats_list, idx, G, wtrue, b_shift):
    """stats_list: per-core [6, PAIRS*512] f32 -> scalar loss (f64 math)."""
    # Sg[q, m, k]: per-pair stats for all K columns
    Sg = np.empty((PAIRS, NW, K), np.float64)
    for c, s in enumerate(stats_list):
        Sg[:, :, c * SLAB : (c + 1) * SLAB] = (
            np.asarray(s, np.float64).reshape(NW, PAIRS, SLAB).transpose(1, 0, 2)
        )
    C = np.concatenate(
        [np.zeros((1, NW, K)), np.cumsum(Sg, axis=0)], axis=0
    )  # [PAIRS+1, 6, K]

    kk = np.arange(K)
    qk = (idx // ROWS_PER_PAIR).astype(np.int64)
    L_raw = C[qk, :, kk]                        # [K, 6] full pairs below boundary
    U_raw = C[PAIRS, :, kk] - C[qk + 1, :, kk]  # full pairs above boundary

    def unpack(raw):
        cnt = raw[:, 0]
        poscnt = raw[:, 1]
        pl = raw[:, 2] + raw[:, 3] / SC
        ev = raw[:, 4] + raw[:, 5] / SC
        return cnt, poscnt, pl, ev

    # exact f64 partial for the boundary pair (256 rows containing idx[k])
    rows = qk[None, :] * ROWS_PER_PAIR + np.arange(ROWS_PER_PAIR)[:, None]  # [256,K]
    gpair = G[rows, kk[None, :]].astype(np.float64)
    low = (rows < idx[None, :]).astype(np.float64)
    glo = gpair * low
    ghi = gpair - glo

    def partial(gm):
        return [
            gm.sum(axis=0),
            (gm * wtrue[rows, 1]).sum(axis=0),
            (gm * wtrue[rows, 2]).sum(axis=0),
            (gm * wtrue[rows, 3]).sum(axis=0),
        ]

    def side(raw, gm):
        cnt, poscnt, pl, ev = unpack(raw)
        pc, pp, ppl, pe = partial(gm)
        cnt = cnt + pc
        poscnt = poscnt + pp
        pl = pl + ppl
        ev = ev + pe
        valid = (cnt > 0.5) & (np.abs(poscnt - 1.0) < 0.25)
        lse = np.log(np.where(valid, np.maximum(ev, 1e-300), 1.0)) + b_shift
        return np.where(valid, (lse - pl) / np.maximum(cnt, 1.0), 0.0).sum()

    return np.array(side(L_raw, glo) + side(U_raw, ghi), dtype=np.float32)


def _ensure_axon_hooks_stub():
    """bass_utils imports antenv.axon_hooks when tracing is requested via
    env; the module is absent on some images. Provide a no-op stub so the
    import never crashes (hook=None -> bass_utils skips tracing)."""
    import sys
    import types

    try:
        import antenv.axon_hooks  # noqa: F401
    except ImportError:
        mod = types.ModuleType("antenv.axon_hooks")
        state = {"hook": None}
        mod.set_axon_ntff_profile_hook = lambda h: state.__setitem__("hook", h)
        mod.get_axon_ntff_profile_hook = lambda: state["hook"]
        sys.modules["antenv.axon_hooks"] = mod


def _device_stats(in_maps):
    _ensure_axon_hooks_stub()
    from concourse.bass_utils import run_bass_kernel_spmd

    if "nc" not in _BASS_CACHE:
        _BASS_CACHE["nc"] = _build_bass()
    last_exc = None
    for attempt in range(4):
        try:
            res = run_bass_kernel_spmd(
                _BASS_CACHE["nc"], in_maps, core_ids=list(range(NCORES))
            )
            return [r["stats"] for r in res.results]
        except Exception as e:  # transient NRT/accelerator hiccups
            last_exc = e
            try:
                # a fresh PJRT client usually recovers a transiently
                # "unrecoverable" accelerator; mirrors a process restart
                import jax
                import jax.extend.backend as _jeb

                jax.clear_caches()
                _jeb.clear_backends()
            except Exception:
                pass
            import time

            time.sleep(2.0 * (attempt + 1))
    raise last_exc


def _sim_stats(in_maps):
    """Numpy emulation of the device kernel (same inputs), for logic validation."""
    outs = []
    for m in in_maps:
        # [NCH, 128, CHUNK, 512] -> [128, 64, 512]
        gm = m["g"].astype(np.float32).transpose(1, 0, 2, 3).reshape(P, NT, SLAB)
        wm = m["w"].astype(np.float32)[:, :, :NW]  # [128, 64, 6]
        acc = np.zeros((NW, PAIRS, SLAB), np.float32)
        for q in range(PAIRS):
            for j in (2 * q, 2 * q + 1):
                acc[:, q, :] += wm[:, j, :].T @ gm[:, j, :]
        outs.append(acc.reshape(NW, PAIRS * SLAB))
    return outs


def _prep(outputs, targets, node_adj, idx_node):
    wmat, wtrue, b_shift = _host_prep(outputs, targets)
    G, idx = _gather_columns(node_adj, idx_node)
    in_maps = [{"g": _build_shard(G, d), "w": wmat} for d in range(NCORES)]
    return in_maps, (idx, G, wtrue, b_shift)


def kernel(outputs, targets, node_adj, idx_node, _simulate=False):
    in_maps, ctx = _prep(outputs, targets, node_adj, idx_node)
    stats = _sim_stats(in_maps) if _simulate else _device_stats(in_maps)
    return _combine(stats, *ctx)
